# revision 34
# baseline (speedup 1.0000x reference)
"""Trainium2 Bass kernel for the LSTM-attention module.

Math (per batch b):
    Wsum = Wa_w + Ua_w, bsum = Wa_b + Ua_b   (host-side, tiny)
    pre[t, o]  = sum_h x[b,t,h] * Wsum[o,h] + bsum[o]
    score[t]   = sum_o tanh(pre[t,o]) * v[o]          (Va_b cancels in softmax)
    attn       = softmax(score)
    context[h] = sum_t attn[t] * x[b,t,h]

Sharding: data-parallel over batch B across 8 NeuronCores (16 batches/core).
Weights replicated. Each core reads its x shard from HBM exactly once; the
per-batch x stays SBUF-resident (float32r) and serves both the transposed
GEMM pass and the natural-layout context pass.

All matmuls run in float32r (full PE rate at free-dim >= 256, ~1e-4 rel
precision). x is transposed on-chip via PE transpose-mode (contraction on
the tensor engine is always over the partition dim, and x arrives
[t, h]-major). The per-batch attention row is transposed for the context
matvec by bouncing it through the `att` DRAM output (SBUF->SBUF
partition-rearrange DMA is broken on TRN2; DRAM->SBUF strided reads are
exact).
"""

import os
import sys

for _p in ("/opt/trn_rl_repo", "/root/.axon_site/_ro/trn_rl_repo"):
    if os.path.isdir(_p) and _p not in sys.path:
        sys.path.insert(0, _p)

import numpy as np

B, T, H = 128, 2048, 512
N_CORES = 8
B_C = B // N_CORES  # batches per core
P = 128

_cache: dict = {}


def _build(b_c: int = B_C, t: int = T):
    import concourse.tile as tile
    from concourse import bacc, mybir
    from concourse.masks import make_identity

    DT = mybir.dt
    F32 = DT.float32
    F32R = DT.float32r
    TANH = mybir.ActivationFunctionType.Tanh
    EXP = mybir.ActivationFunctionType.Exp

    KO = H // P   # 4 contraction chunks
    OG = H // P   # 4 output (o) chunks
    NT = 512      # bt-group width (= max fp32 moving free dim = 1 PSUM bank)
    GPB = t // NT   # groups per batch
    TCB = t // P    # t-chunks per batch
    assert t % NT == 0

    nc = bacc.Bacc(
        "TRN2", target_bir_lowering=False, debug=False, num_devices=N_CORES
    )
    x_d = nc.dram_tensor("x", [b_c * t, H], F32R, kind="ExternalInput").ap()
    wT_d = nc.dram_tensor("wT", [H, H], F32R, kind="ExternalInput").ap()
    bT_d = nc.dram_tensor("bsumT", [P, OG], F32, kind="ExternalInput").ap()
    vT_d = nc.dram_tensor("vT", [P, OG], F32R, kind="ExternalInput").ap()
    ctx_d = nc.dram_tensor("ctx", [b_c, H], F32, kind="ExternalOutput").ap()
    att_d = nc.dram_tensor("att", [b_c, t], F32, kind="ExternalOutput").ap()
    esc_d = nc.dram_tensor("esc", [b_c, t], F32, kind="ExternalOutput").ap()

    with tile.TileContext(nc) as tc:
        with (
            tc.tile_pool(name="consts", bufs=1) as consts,
            tc.tile_pool(name="xnat", bufs=2) as xpool,
            tc.tile_pool(name="xt", bufs=3 * KO) as xtpool,
            tc.tile_pool(name="th", bufs=3 * OG) as thpool,
            tc.tile_pool(name="rows", bufs=2) as rows,
            tc.tile_pool(name="small", bufs=3) as small,
            tc.tile_pool(name="pxt", bufs=4, space="PSUM") as pxt_pool,
            tc.tile_pool(name="ppre", bufs=2, space="PSUM") as ppre_pool,
            tc.tile_pool(name="psc", bufs=1, space="PSUM") as psc_pool,
            tc.tile_pool(name="pctx", bufs=1, space="PSUM") as pctx_pool,
        ):
            # ---- one-time constants ----
            ident_f = consts.tile([P, P], F32)
            make_identity(nc, ident_f[:])
            ident = consts.tile([P, P], F32R)
            nc.vector.tensor_copy(ident[:], ident_f[:])

            # Wsum^T as [p=h_inner, k=h_outer, o], rounded to f32r by DMA cast
            wTr = consts.tile([P, KO, H], F32R)
            nc.scalar.dma_start(
                wTr[:], wT_d.rearrange("(k p) o -> p k o", p=P)
            )
            bsumT = consts.tile([P, OG], F32)
            nc.scalar.dma_start(bsumT[:], bT_d[:])
            vT = consts.tile([P, OG], F32R)
            nc.scalar.dma_start(vT[:], vT_d[:])

            for b in range(b_c):
                # natural-layout x for this batch: [p, t_chunk, h]
                x_nat = xpool.tile([P, TCB, H], F32R, tag="xnat")
                e_b = rows.tile([1, t], F32, tag="e")
                s_parts = small.tile([1, GPB], F32, tag="sparts")
                pctx = pctx_pool.tile([1, H], F32, tag="pctx")
                for q in range(GPB):
                    g = b * GPB + q
                    if b == 0 and q == 0:
                        # split the very first load so the PE can start sooner
                        for c in range(NT // P):
                            nc.sync.dma_start(
                                x_nat[:, c : c + 1, :],
                                x_d[g * NT + c * P : g * NT + (c + 1) * P, :]
                                .rearrange("(c p) h -> p c h", p=P),
                            )
                    else:
                        nc.sync.dma_start(
                            x_nat[:, q * (NT // P) : (q + 1) * (NT // P), :],
                            x_d[g * NT : (g + 1) * NT, :].rearrange(
                                "(c p) h -> p c h", p=P
                            ),
                        )
                    # ---- transpose x group -> xT[k] = [h_k, bt] ----
                    xts = []
                    for k in range(KO):
                        pxt = pxt_pool.tile([P, NT], F32R, tag="pxt")
                        for j in range(NT // P):
                            nc.tensor.transpose(
                                pxt[:, j * P : (j + 1) * P],
                                x_nat[:, q * (NT // P) + j, k * P : (k + 1) * P],
                                ident[:],
                            )
                        xt_k = xtpool.tile([P, NT], F32R, tag="xt")
                        nc.vector.tensor_copy(xt_k[:], pxt[:])
                        xts.append(xt_k)
                    # ---- GEMM + tanh per o-chunk ----
                    ths = []
                    for og in range(OG):
                        ppre = ppre_pool.tile([P, NT], F32, tag="ppre")
                        for k in range(KO):
                            nc.tensor.matmul(
                                ppre[:],
                                wTr[:, k, og * P : (og + 1) * P],
                                xts[k][:],
                                start=(k == 0),
                                stop=(k == KO - 1),
                            )
                        th = thpool.tile([P, NT], F32R, tag="th")
                        nc.scalar.activation(
                            th[:], ppre[:], TANH, bias=bsumT[:, og : og + 1]
                        )
                        ths.append(th)
                    # ---- score matvec: [1, NT] ----
                    psc = psc_pool.tile([1, NT], F32, tag="psc")
                    for og in range(OG):
                        nc.tensor.matmul(
                            psc[:],
                            vT[:, og : og + 1],
                            ths[og][:],
                            start=(og == 0),
                            stop=(og == OG - 1),
                        )
                    # exp of this score quarter (no max-sub: |score|<=sum|v|~20)
                    nc.scalar.activation(
                        e_b[:, q * NT : (q + 1) * NT], psc[:], EXP,
                        accum_out=s_parts[:, q : q + 1],
                    )
                    # bounce unnormalized weights through DRAM to transpose
                    nc.sync.dma_start(
                        esc_d[b : b + 1, q * NT : (q + 1) * NT],
                        e_b[:, q * NT : (q + 1) * NT],
                    )
                    wTq = small.tile([P, NT // P], F32R, tag="wtq")
                    with nc.allow_non_contiguous_dma(
                        reason="per-group attention-row transpose readback"
                    ):
                        nc.sync.dma_start(
                            wTq[:],
                            esc_d[b, q * NT : (q + 1) * NT]
                            .bitcast(F32R)
                            .rearrange("(c p) -> p c", p=P),
                        )
                    # unnormalized context accumulation for this group
                    for j in range(NT // P):
                        c = q * (NT // P) + j
                        nc.tensor.matmul(
                            pctx[:],
                            wTq[:, j : j + 1],
                            x_nat[:, c, :],
                            start=(c == 0),
                            stop=(c == TCB - 1),
                        )

                # ---- normalize: rec = 1 / sum(e) ----
                if GPB == 1:
                    s_acc = s_parts
                else:
                    s_pair = small.tile([1, GPB // 2], F32, tag="spair")
                    nc.vector.tensor_add(
                        s_pair[:], s_parts[:, 0 : GPB // 2], s_parts[:, GPB // 2 :]
                    )
                    s_acc = small.tile([1, 1], F32, tag="sacc")
                    nc.vector.reduce_sum(
                        s_acc[:], s_pair[:], axis=mybir.AxisListType.X
                    )
                rec = small.tile([1, 1], F32, tag="rec")
                nc.vector.reciprocal(rec[:], s_acc[:])
                att_row = rows.tile([1, t], F32, tag="att")
                nc.vector.tensor_scalar_mul(att_row[:], e_b[:], rec[:])
                nc.sync.dma_start(att_d[b : b + 1, :], att_row[:])
                ctx_row = small.tile([1, H], F32, tag="ctxrow")
                nc.vector.tensor_scalar_mul(ctx_row[:], pctx[:], rec[:])
                nc.sync.dma_start(ctx_d[b : b + 1, :], ctx_row[:])

    nc.compile()
    return nc


def _prep_weights(Wa_w, Wa_b, Ua_w, Ua_b, Va_w):
    Wsum = np.asarray(Wa_w, dtype=np.float32) + np.asarray(Ua_w, dtype=np.float32)
    wT = np.ascontiguousarray(Wsum.T)
    bsum = np.asarray(Wa_b, dtype=np.float32) + np.asarray(Ua_b, dtype=np.float32)
    bsumT = np.ascontiguousarray(bsum.reshape(H // P, P).T)
    v = np.asarray(Va_w, dtype=np.float32).reshape(-1)
    vT = np.ascontiguousarray(v.reshape(H // P, P).T)
    return wT, bsumT, vT


def kernel(lstm_output, Wa_w, Wa_b, Ua_w, Ua_b, Va_w, Va_b):
    from concourse.bass_utils import run_bass_kernel_spmd

    if "nc" not in _cache:
        _cache["nc"] = _build()
    nc = _cache["nc"]

    wT, bsumT, vT = _prep_weights(Wa_w, Wa_b, Ua_w, Ua_b, Va_w)
    x = np.asarray(lstm_output, dtype=np.float32)

    in_maps = []
    for c in range(N_CORES):
        xs = np.ascontiguousarray(x[c * B_C : (c + 1) * B_C].reshape(B_C * T, H))
        in_maps.append({"x": xs, "wT": wT, "bsumT": bsumT, "vT": vT})

    res = run_bass_kernel_spmd(nc, in_maps, core_ids=list(range(N_CORES)))
    ctx = np.concatenate([res.results[c]["ctx"] for c in range(N_CORES)], axis=0)
    att = np.concatenate([res.results[c]["att"] for c in range(N_CORES)], axis=0)
    return ctx, att
